# revision 1
# baseline (speedup 1.0000x reference)
"""Trainium2 Bass kernel for DeformablePatchSampler2d.

out[n, m, c, i, j] = bilinear_sample(x[n, c], row=RY[m, j], col=CX[m, i])

The sampling grid is batch/channel-invariant and known on the host from
`offset`, so all gather indices become static DMA access patterns baked in
at build time. Bilinear is separable: stage 1 interpolates image rows
(3-tap, weights per (m, j)), stage 2 interpolates image cols (per (m, i)).

Layout: 2 patches per "stack" on 128 partitions (partition = (s, c)),
all compute as f32 tensor_tensor ops on VectorE/GpSimdE with zero-stride
broadcast weight APs. Data-parallel over batch N=8 across 8 NeuronCores
(same NEFF, per-core x slice).
"""
import numpy as np

_P = 16
_NPH = _NPW = 8
_M = 64
_H = _W = 384
_C = 64
_N = 8
_NS = 32          # stacks of 2 patches
_WPS = 96         # weight floats per stack: 2 stages * 3 taps * 16
_GPSIMD_EVERY = 3  # stack k -> gpsimd if k % _GPSIMD_EVERY == _GPSIMD_EVERY-1


def _precompute(offset: np.ndarray):
    """Window origins + 3-tap weights, f32 coord math mirroring the reference."""
    offset = offset.astype(np.float32)
    one, half = np.float32(1.0), np.float32(0.5)
    ch = np.linspace(0.0, float(_H), _NPH + 4).astype(np.float32)[2:-2]
    cw = np.linspace(0.0, float(_W), _NPW + 4).astype(np.float32)[2:-2]
    rel = np.arange(_P, dtype=np.float32) - np.float32(_P // 2)
    a = np.arange(_M) // _NPW
    b = np.arange(_M) % _NPW
    hc = ch[a][:, None] + rel[None, :]
    wcen = cw[b][:, None] + rel[None, :]
    gx = (np.float32(2.0) * hc / np.float32(_H - 1) - one) + offset[:, 0:1]
    gy = (np.float32(2.0) * wcen / np.float32(_W - 1) - one) + offset[:, 1:2]
    CX = (((gx + one) * np.float32(_W) - one) * half).astype(np.float64)  # (M,16) cols, dim i
    RY = (((gy + one) * np.float32(_H) - one) * half).astype(np.float64)  # (M,16) rows, dim j

    r0 = np.floor(RY[:, 0]).astype(np.int64)
    c0 = np.floor(CX[:, 0]).astype(np.int64)
    t_r = RY - (r0[:, None] + np.arange(_P)[None, :])
    t_c = CX - (c0[:, None] + np.arange(_P)[None, :])
    assert (t_r >= 0).all() and (t_r < 2).all()
    assert (t_c >= 0).all() and (t_c < 2).all()
    assert r0.min() >= 0 and (r0 + 17).max() <= _H - 1
    assert c0.min() >= 0 and (c0 + 17).max() <= _W - 1

    def taps(t):
        w0 = np.maximum(0.0, 1.0 - t)
        w2 = np.maximum(0.0, t - 1.0)
        return np.stack([w0, 1.0 - w0 - w2, w2], axis=-1).astype(np.float32)

    wr = taps(t_r)  # (M, 16, 3) applies to j (rows)
    wc = taps(t_c)  # (M, 16, 3) applies to i (cols)
    nt_r = np.where(np.abs(wr[:, :, 2]).max(axis=1) > 0, 3, 2)
    nt_c = np.where(np.abs(wc[:, :, 2]).max(axis=1) > 0, 3, 2)
    return r0, c0, wr, wc, nt_r, nt_c


def _plan(offset: np.ndarray):
    """Pair patches into stacks (same tap profile together) + pack weights."""
    r0, c0, wr, wc, nt_r, nt_c = _precompute(offset)
    order = np.lexsort((np.arange(_M), nt_c, nt_r))  # (M,) patch ids
    w_all = np.zeros((128, _NS * _WPS), dtype=np.float32)
    stacks = []
    for k in range(_NS):
        ms = [int(order[2 * k]), int(order[2 * k + 1])]
        rt = int(max(nt_r[ms[0]], nt_r[ms[1]]))
        ct = int(max(nt_c[ms[0]], nt_c[ms[1]]))
        for s, m in enumerate(ms):
            rows = slice(s * 64, (s + 1) * 64)
            base = k * _WPS
            for jk in range(3):
                w_all[rows, base + jk * 16:base + jk * 16 + 16] = wr[m, :, jk][None, :]
            for ik in range(3):
                w_all[rows, base + 48 + ik * 16:base + 48 + ik * 16 + 16] = wc[m, :, ik][None, :]
        stacks.append(dict(ms=ms, rt=rt, ct=ct,
                           r0=[int(r0[m]) for m in ms], c0=[int(c0[m]) for m in ms]))
    return stacks, w_all


def _build(stacks):
    import concourse.bacc as bacc
    import concourse.mybir as mybir
    from concourse.bass import AP
    from concourse.tile import TileContext

    f32 = mybir.dt.float32
    mult = mybir.AluOpType.mult
    add = mybir.AluOpType.add

    nc = bacc.Bacc("TRN2", target_bir_lowering=False)
    x_n = nc.dram_tensor("x_n", (_C, _H, _W), f32, kind="ExternalInput")
    w_d = nc.dram_tensor("w_all", (128, _NS * _WPS), f32, kind="ExternalInput")
    out_n = nc.dram_tensor("out_n", (_M, _C, _P, _P), f32, kind="ExternalOutput")

    def sub_ap(base_ap, extra_off, free_dims):
        return AP(base_ap.tensor, base_ap.offset + extra_off,
                  [list(base_ap.ap[0])] + [list(d) for d in free_dims])

    with TileContext(nc) as tc:
        with tc.tile_pool(name="wpool", bufs=1) as wpool, \
             tc.tile_pool(name="apool", bufs=4) as apool, \
             tc.tile_pool(name="tpool", bufs=3) as tpool, \
             tc.tile_pool(name="mpool", bufs=4) as mpool, \
             tc.tile_pool(name="opool", bufs=3) as opool:
            W_sb = wpool.tile([128, _NS * _WPS], f32)
            nc.sync.dma_start(out=W_sb[:], in_=w_d[:])
            wb = W_sb[:]
            for k, st in enumerate(stacks):
                eng = nc.gpsimd if k % _GPSIMD_EVERY == _GPSIMD_EVERY - 1 else nc.vector
                A = apool.tile([128, 324], f32)
                for s in range(2):
                    r0, c0 = st["r0"][s], st["c0"][s]
                    nc.sync.dma_start(
                        out=A[s * 64:(s + 1) * 64, :],
                        in_=x_n[:, r0:r0 + 18, c0:c0 + 18])
                ab, base = A[:], k * _WPS
                # stage 1: T[p, (j, q)] = sum_jk wr[jk, j] * A[p, (j+jk, q)]
                T = tpool.tile([128, 288], f32)
                tb = T[:]
                t_ap = sub_ap(tb, 0, [[18, 16], [1, 18]])
                for jk in range(st["rt"]):
                    a_ap = sub_ap(ab, jk * 18, [[18, 16], [1, 18]])
                    w_ap = sub_ap(wb, base + jk * 16, [[1, 16], [0, 18]])
                    if jk == 0:
                        eng.tensor_tensor(out=t_ap, in0=a_ap, in1=w_ap, op=mult)
                    else:
                        Mt = mpool.tile([128, 288], f32)
                        m_ap = sub_ap(Mt[:], 0, [[18, 16], [1, 18]])
                        eng.tensor_tensor(out=m_ap, in0=a_ap, in1=w_ap, op=mult)
                        eng.tensor_tensor(out=t_ap, in0=t_ap, in1=m_ap, op=add)
                # stage 2: O[p, (i, j)] = sum_ik wc[ik, i] * T[p, (j, i+ik)]
                O = opool.tile([128, 256], f32)
                ob = O[:]
                o_ap = sub_ap(ob, 0, [[16, 16], [1, 16]])
                for ik in range(st["ct"]):
                    t2_ap = sub_ap(tb, ik, [[1, 16], [18, 16]])
                    w_ap = sub_ap(wb, base + 48 + ik * 16, [[1, 16], [0, 16]])
                    if ik == 0:
                        eng.tensor_tensor(out=o_ap, in0=t2_ap, in1=w_ap, op=mult)
                    else:
                        Mt = mpool.tile([128, 288], f32)
                        m_ap = sub_ap(Mt[:], 0, [[16, 16], [1, 16]])
                        eng.tensor_tensor(out=m_ap, in0=t2_ap, in1=w_ap, op=mult)
                        eng.tensor_tensor(out=o_ap, in0=o_ap, in1=m_ap, op=add)
                for s in range(2):
                    nc.scalar.dma_start(
                        out=out_n[st["ms"][s]],
                        in_=O[s * 64:(s + 1) * 64, :])
    nc.compile()
    return nc


def _run(nc, x, w_all, **kwargs):
    from concourse.bass_utils import run_bass_kernel_spmd
    in_maps = [{"x_n": np.ascontiguousarray(x[n]), "w_all": w_all}
               for n in range(_N)]
    return run_bass_kernel_spmd(nc, in_maps, core_ids=list(range(_N)), **kwargs)


def kernel(x: np.ndarray, offset: np.ndarray, _bench=[None]) -> np.ndarray:
    x = np.asarray(x, dtype=np.float32)
    offset = np.asarray(offset, dtype=np.float32)
    stacks, w_all = _plan(offset)
    nc = _build(stacks)
    res = _run(nc, x, w_all)
    return np.stack([res.results[n]["out_n"] for n in range(_N)])
